# revision 1
# baseline (speedup 1.0000x reference)
"""Trainium2 Bass kernel for RoPE linear attention (no softmax, strict causal).

Computes: QR = rope(Q); S = tril(QR @ QR^T, -1); out = S @ V
for Q [B=2, H=8, T=2048, N=1024], V [B,H,T,D=128], K == Q.

Sharding: B*H = 16 (b,h) pairs -> 2 per core across 8 cores (fully parallel).

Per-core device pipeline (per (b,h)):
  1. load Q tiles fp32 [128, 1024]
  2. cast fp32->fp16 with pair de-interleave (evens -> cols 0:512, odds -> rest)
  3. PE transpose (fp16, grouped 4 chunks/psum bank) -> QRT [n', t]
     as one tensor qrt_big[:, k*T:(k+1)*T] = chunk k
  4. rope in transposed layout (unit-stride fp16 tensor_tensor, in place)
  5. scores: P[a,b] = QRT[:,a]^T @ QRT[:,b] for a <= b only (fp16, fp32 PSUM).
     By symmetry of S, P[a,b] is exactly the lhsT ([s, t]) the AV matmul needs.
  6. PSUM -> SBUF fp16 (diagonal blocks masked to strict-upper = s < t)
  7. AV: out[b] = sum_{a<=b} P[a,b] @ V[a], fp32 PSUM accum -> fp32 out

Scores and AV are each split by b-half and interleaved (S0h0, rope0h1,
AV0[b<8], S0h1 (+bh1 transposes injected into the PE stream), rope1h0,
AV0[b>=8], S1h0, rope1h1, AV1[b<8], S1h1, AV1[b>=8]) so that on every
in-order engine/ring no phase queues behind work it doesn't depend on:
rope never blocks PSUM-releasing drains on DVE, AV fills the gap while
the next rope half completes, and the second head's pipeline rides the
first head's compute.
"""

import math
import os
import sys

import numpy as np

for _p in ("/opt/trn_rl_repo",):
    if _p not in sys.path and os.path.isdir(_p):
        sys.path.insert(0, _p)

THETA = 2 ** 16
B, H, T, N, D = 2, 8, 2048, 1024, 128
NB = T // 128          # 16 t-blocks
NC_COUNT = 8
BH_PER_CORE = (B * H) // NC_COUNT  # 2
NPAIR = N // 2         # 512 rotation pairs
NCHUNK = N // 128      # 8 partition chunks of QRT
NPCH = NPAIR // 128    # 4 pair chunks

_cache = {}


def _make_tables():
    """cos/sin tables in transposed, pair-collapsed layout [512, 2048] fp16.

    Phase arithmetic replicates reference._get_freqs/_rope bit-for-bit in fp32
    (jnp ops on CPU), so the only table error is the final fp16 quantization.
    """
    import jax
    import jax.numpy as jnp

    with jax.default_device(jax.devices("cpu")[0]):
        pos = jnp.floor(jnp.arange(N, dtype=jnp.float32) / 2.0) * 2.0
        freqs = 1.0 / (THETA ** (pos / N)) / (2.0 * math.pi)        # (N,) fp32
        r_phases = jnp.arange(T, dtype=jnp.float32)[:, None] * freqs[None, :]
        ph = (r_phases % 1.0) * (2.0 * math.pi)
        c = np.asarray(jnp.cos(ph))                                  # (T, N) fp32
        s = np.asarray(jnp.sin(ph))
    ct = np.ascontiguousarray(c[:, 0::2].T).astype(np.float16)
    st = np.ascontiguousarray(s[:, 0::2].T).astype(np.float16)
    return ct, st


def _build_nc():
    import concourse.mybir as mybir
    from concourse import bacc
    from concourse.tile import TileContext

    f32 = mybir.dt.float32
    f16 = mybir.dt.float16

    ct_np, st_np = _make_tables()
    # mask[j, i] = 1 if j < i else 0 (keep strictly-upper: s < t)
    mask_np = np.triu(np.ones((128, 128), np.float16), 1)
    ident_np = np.eye(128, dtype=np.float16)

    nc = bacc.Bacc("TRN2", target_bir_lowering=False, debug=False,
                   num_devices=NC_COUNT)
    q = nc.dram_tensor("q", [BH_PER_CORE, T, N], f32, kind="ExternalInput")
    v = nc.dram_tensor("v", [BH_PER_CORE, T, D], f32, kind="ExternalInput")
    out = nc.dram_tensor("out", [BH_PER_CORE, T, D], f32, kind="ExternalOutput")
    ct_dram = nc.inline_tensor(ct_np, name="ct_tab")
    st_dram = nc.inline_tensor(st_np, name="st_tab")
    mask_dram = nc.inline_tensor(mask_np, name="mask_tab")
    ident_dram = nc.inline_tensor(ident_np, name="ident_tab")

    with TileContext(nc) as tc:
        with tc.tile_pool(name="const", bufs=1) as cpool, \
             tc.tile_pool(name="work", bufs=1) as pool, \
             tc.tile_pool(name="psS", bufs=4, space="PSUM") as psS, \
             tc.tile_pool(name="psT", bufs=2, space="PSUM") as psT, \
             tc.tile_pool(name="psO", bufs=2, space="PSUM") as psO:

            # constants: tiles allocated now, table DMAs emitted just-in-
            # time between the Q loads (so they don't delay the fill)
            ct_sb = [cpool.tile([128, T], f16, name=f"ct{j}")
                     for j in range(NPCH)]
            st_sb = [cpool.tile([128, T], f16, name=f"st{j}")
                     for j in range(NPCH)]
            mask_sb = cpool.tile([128, 128], f16, name="mask")
            nc.sync.dma_start(out=mask_sb, in_=mask_dram[:, :])
            ident_sb = cpool.tile([128, 128], f16, name="ident")
            nc.sync.dma_start(out=ident_sb, in_=ident_dram[:, :])

            def load_tables(js):
                for j in js:
                    nc.sync.dma_start(out=ct_sb[j],
                                      in_=ct_dram[j * 128:(j + 1) * 128, :])
                    nc.sync.dma_start(out=st_sb[j],
                                      in_=st_dram[j * 128:(j + 1) * 128, :])

            copy_alt = [0]  # round-robin ACT/DVE for PSUM drains

            def drain_copy(dst, src):
                if copy_alt[0] % 2 == 0:
                    nc.scalar.copy(dst, src)
                else:
                    nc.vector.tensor_copy(out=dst, in_=src)
                copy_alt[0] += 1

            TH = T // 2

            def load_cast(bh, split_cast, hook=None):
                """V load + Q load + cast. Returns (vf, qrt views, qd list)."""
                vf = pool.tile([128, NB * 128], f16, tag="vf", bufs=2,
                               name=f"vf{bh}")
                nc.gpsimd.dma_start(
                    out=vf.rearrange("p (a d) -> p a d", a=NB),
                    in_=v[bh].rearrange("(a p) d -> p a d", p=128),
                )
                qrt_big = pool.tile([128, NCHUNK * T], f16, tag="qrt", bufs=2,
                                    name=f"qrtbig{bh}")
                qrt = [qrt_big[:, k * T:(k + 1) * T] for k in range(NCHUNK)]
                qrt_3d = qrt_big.rearrange("p (c t) -> p c t", c=NCHUNK)
                qds = []
                for tt in range(NB):
                    qf = pool.tile([128, N], f32, tag="qstage", bufs=4,
                                   name=f"qf{bh}_{tt}")
                    nc.sync.dma_start(out=qf,
                                      in_=q[bh, tt * 128:(tt + 1) * 128, :])
                    qd = pool.tile([128, N], f16, tag="qde", bufs=10,
                                   name=f"qd{bh}_{tt}")
                    # plain unit-stride cast (keeps interleaved order; the
                    # PE transpose de-interleaves via stride-2 input APs)
                    nc.scalar.copy(qd, qf)
                    qds.append(qd)
                    if hook is not None:
                        hook(tt)
                return vf, qrt, qrt_3d, qds

            def emit_transpose(bh, qrt_3d, qds, tt):
                """PE transposes of tile tt, 4 chunks per PSUM bank.

                De-interleaves via stride-2 input APs: chunk k<4 takes even
                source columns of pair block k, chunk k>=4 the odd columns,
                so chunk k partition m is rotation pair k%4*128+m as before.
                """
                qd = qds[tt]
                pt = psT.tile([128, 1024], f16, tag="pt",
                              name=f"pt{bh}_{tt}")
                for k in range(NCHUNK):
                    j = k % NPCH
                    par = k // NPCH              # 0 = even half, 1 = odd
                    nc.tensor.transpose(
                        pt[:, k * 128:(k + 1) * 128],
                        qd[:, j * 256 + par: (j + 1) * 256: 2],
                        ident_sb)
                drain_copy(
                    qrt_3d[:, :, tt * 128:(tt + 1) * 128],
                    pt.rearrange("p (c t) -> p c t", c=NCHUNK))

            def emit_rope(bh, qrt, h):
                hsl = slice(h * TH, (h + 1) * TH)
                for j in range(NPCH):
                    qe, qo = qrt[j][:, hsl], qrt[j + NPCH][:, hsl]
                    c_t, s_t = ct_sb[j][:, hsl], st_sb[j][:, hsl]
                    t1 = pool.tile([128, TH], f16, tag="tmp1", bufs=2,
                                   name=f"t1_{bh}_{j}_{h}")
                    t2 = pool.tile([128, TH], f16, tag="tmp2", bufs=2,
                                   name=f"t2_{bh}_{j}_{h}")
                    nc.vector.tensor_mul(out=t1, in0=qe, in1=s_t)
                    nc.vector.tensor_mul(out=t2, in0=qo, in1=s_t)
                    nc.vector.tensor_mul(out=qe, in0=qe, in1=c_t)
                    nc.vector.tensor_sub(out=qe, in0=qe, in1=t2)
                    nc.vector.tensor_mul(out=qo, in0=qo, in1=c_t)
                    nc.vector.tensor_add(out=qo, in0=qo, in1=t1)

            korder = [0, 4, 1, 5, 2, 6, 3, 7]
            HB = NB // 2

            def alloc_strips(bh):
                return [
                    pool.tile([128, (NB - a) * 128], f16, tag=f"strip{a}",
                              bufs=1, name=f"strip{bh}_{a}")
                    for a in range(NB)
                ]

            def phase_s_half(bh, qrt, strips, hpass, inject=None):
                """Scores P[a,b] for b in one half (pass 0: b<=7, pass 1:
                b>=8). With korder [0,4,1,5,...] each rope step (pair j,
                half h) readies chunks j and j+4, so pass-0 matmuls unlock
                while rope is still running."""
                for a in range(NB):
                    if inject is not None:
                        inject(a)
                    blo = a if hpass == 0 else max(a, HB)
                    bhi = HB if hpass == 0 else NB
                    if blo >= bhi:
                        continue
                    strip = strips[a]
                    asl = slice(a * 128, (a + 1) * 128)
                    groups = []
                    for gs in range(blo, bhi, 4):
                        w = min(4, bhi - gs) * 128
                        ps = psS.tile([128, 512], f32, tag="ps",
                                      name=f"ps{bh}_{hpass}_{a}_{gs}")
                        groups.append((gs, w, ps))
                    for ki, k in enumerate(korder):
                        for (gs, w, ps) in groups:
                            nc.tensor.matmul(
                                ps[:, :w],
                                lhsT=qrt[k][:, asl],
                                rhs=qrt[k][:, gs * 128: gs * 128 + w],
                                start=(ki == 0),
                                stop=(ki == NCHUNK - 1),
                            )
                    for (gs, w, ps) in groups:
                        off = (gs - a) * 128      # strip column offset
                        if gs == a:
                            # diagonal block: strict-upper mask (s < t)
                            nc.vector.tensor_mul(
                                out=strip[:, off:off + 128],
                                in0=ps[:, 0:128], in1=mask_sb,
                            )
                            if w > 128:
                                nc.scalar.copy(strip[:, off + 128:off + w],
                                               ps[:, 128:w])
                        else:
                            nc.scalar.copy(strip[:, off:off + w], ps[:, :w])

            def phase_av(bh, vf, strips, blo, bhi):
                for b in range(blo, bhi):
                    po = psO.tile([128, D], f32, tag="po", name=f"po{bh}_{b}")
                    for a in range(b + 1):
                        nc.tensor.matmul(
                            po,
                            lhsT=strips[a][:, (b - a) * 128:(b - a + 1) * 128],
                            rhs=vf[:, a * 128:(a + 1) * 128],
                            start=(a == 0),
                            stop=(a == b),
                        )
                    ob = pool.tile([128, D], f32, tag="ostage", bufs=4,
                                   name=f"ob{bh}_{b}")
                    nc.vector.tensor_copy(out=ob, in_=po)
                    nc.sync.dma_start(out=out[bh, b * 128:(b + 1) * 128, :],
                                      in_=ob)

            # bh0: self-paced pipeline (PE idle during fill anyway)
            def const_hook(tt):
                if tt == 7:
                    load_tables((0, 1))
                elif tt == 11:
                    load_tables((2, 3))

            vf0, qrt0, qrt0_3d, qds0 = load_cast(0, split_cast=False,
                                                 hook=const_hook)
            for tt in range(NB):
                emit_transpose(0, qrt0_3d, qds0, tt)
            emit_rope(0, qrt0, 0)

            # bh1: loads+casts up front; transposes injected into the
            # scores(bh0) PE stream at the loads' pace; rope per half once
            # its 8 tiles are transposed
            vf1, qrt1, qrt1_3d, qds1 = load_cast(1, split_cast=False)

            def inject(a):
                # during S0's h1 pass: late enough that bh1's loads/casts
                # are done; transposes ride the S0 PE stream (rope emitted
                # separately so the pass's diag masks aren't queued behind
                # it on the in-order DVE)
                if 2 <= a <= 9:
                    for tt in (2 * (a - 2), 2 * (a - 2) + 1):
                        emit_transpose(1, qrt1_3d, qds1, tt)

            # AV for b<=7 only needs h0-pass strips: run it in the gap while
            # rope-h1 finishes; same split shortens the kernel tail.
            strips0 = alloc_strips(0)
            phase_s_half(0, qrt0, strips0, 0)
            # rope h1 emitted here so the h0-pass strip drains and diag
            # masks do not queue behind it on the in-order DVE
            emit_rope(0, qrt0, 1)
            phase_av(0, vf0, strips0, 0, HB)
            phase_s_half(0, qrt0, strips0, 1, inject=inject)
            emit_rope(1, qrt1, 0)
            phase_av(0, vf0, strips0, HB, NB)
            strips1 = alloc_strips(1)
            phase_s_half(1, qrt1, strips1, 0)
            emit_rope(1, qrt1, 1)
            phase_av(1, vf1, strips1, 0, HB)
            phase_s_half(1, qrt1, strips1, 1)
            phase_av(1, vf1, strips1, HB, NB)

    nc.compile()
    return nc


def _get_nc():
    if "nc" not in _cache:
        _cache["nc"] = _build_nc()
    return _cache["nc"]


def kernel(Q, K, V):
    from concourse import bass_utils

    del K  # K is Q by construction
    Qr = np.ascontiguousarray(Q.reshape(B * H, T, N), dtype=np.float32)
    Vr = np.ascontiguousarray(V.reshape(B * H, T, D), dtype=np.float32)

    nc = _get_nc()
    in_maps = []
    for c in range(NC_COUNT):
        lo = c * BH_PER_CORE
        in_maps.append({
            "q": np.ascontiguousarray(Qr[lo:lo + BH_PER_CORE]),
            "v": np.ascontiguousarray(Vr[lo:lo + BH_PER_CORE]),
        })

    res = bass_utils.run_bass_kernel_spmd(
        nc, in_maps, core_ids=list(range(NC_COUNT)),
    )
    _cache["last_result"] = res
    outs = [res.results[c]["out"].reshape(BH_PER_CORE, T, D)
            for c in range(NC_COUNT)]
    return np.concatenate(outs, axis=0).reshape(B, H, T, D).astype(np.float32)

